# revision 5
# baseline (speedup 1.0000x reference)
"""DFlashAttention Trainium2 kernel (8 NeuronCores, SPMD, no collectives).

Problem (hardcoded shapes): B=4, QL=1024, CL=3072, KL=4096, H=2048,
NH=16 q-heads, NKV=4 kv-heads, HD=128.

Sharding: core i = (batch b = i//2, head-group g = i%2). Each core computes
8 q-heads / 2 kv-heads for one batch and produces a partial o_proj output
(contraction over its head block of Wo); the host sums the two partials per
batch.

v2 design (vs v1 baseline at 1106us):
  - ALL transposes happen on the HOST: kv^T, cos^T, sin^T and all weights
    are pre-arranged into their exact SBUF layouts, so the PE never runs a
    transpose (was 576 transposes + 576 PSUM->SBUF copies).
  - RMSNorm without PE broadcast or DVE reciprocal: ssq matmul uses a
    [128,128] ones stationary so the partition-sum lands broadcast across
    all 128 partitions; rstd = exp(-0.5*ln(var+eps)) on ACT (Square, Ln,
    Exp and Copy all live in the single natural_log_exp_and_others table
    set - zero table switches in the whole kernel).
  - norm apply fused: qn = (proj * w[d]) * rstd via one scalar_tensor_tensor.
  - ATT: sT tiles are [128,1024] (2 PSUM banks) so exp runs once per
    (head, kt) at FD=1024 (1147ns) under 6 FD-512 matmuls (~1280ns) - the
    ACT engine stays just under the PE.
  - softmax normalize off the critical path: oT PSUM is released by an ACT
    copy; 1/den via DVE reciprocal on [1,512] tiles; broadcast via gpsimd
    partition_broadcast; final multiply on DVE. No PE participation.
"""
import os
import sys

sys.path.insert(0, "/opt/trn_rl_repo")

import numpy as np

import concourse.bass as bass
import concourse.tile as tile
from concourse import bacc, mybir
from concourse.bass_utils import run_bass_kernel_spmd

f32 = mybir.dt.float32
f32r = mybir.dt.float32r
AF = mybir.ActivationFunctionType
ALU = mybir.AluOpType

P = 128
H = 2048
HT = H // P          # 16 h-tiles
QL = 1024
CL = 3072
KL = CL + QL         # 4096
KT_N = KL // P       # 32 k-tiles
HD = 128
NHC = 8              # q heads per core
NKVC = 2             # kv heads per core
SCALE = HD ** -0.5
EPS = 1e-6

_NC = None


def build_nc():
    nc = bacc.Bacc("TRN2", target_bir_lowering=False, debug=False)

    # host-pre-arranged layouts (see _make_in_maps)
    kvT = nc.dram_tensor("kvT", [P, HT, KL], f32r, kind="ExternalInput").ap()
    cosT = nc.dram_tensor("cosT", [P, KL], f32r, kind="ExternalInput").ap()
    sinT = nc.dram_tensor("sinT", [P, KL], f32r, kind="ExternalInput").ap()
    wq = nc.dram_tensor("wq", [P, NHC, HT, HD], f32r, kind="ExternalInput").ap()
    wk = nc.dram_tensor("wk", [P, HT, NKVC * HD], f32r, kind="ExternalInput").ap()
    wv = nc.dram_tensor("wv", [P, HT, NKVC * HD], f32r, kind="ExternalInput").ap()
    wo = nc.dram_tensor("wo", [P, 2, NHC, 1024], f32r, kind="ExternalInput").ap()
    qnw = nc.dram_tensor("qnw", [P, 1], f32r, kind="ExternalInput").ap()
    knw = nc.dram_tensor("knw", [P, 1], f32r, kind="ExternalInput").ap()
    out = nc.dram_tensor("out", [QL, H], f32, kind="ExternalOutput").ap()

    # HBM staging for V (saves SBUF during the KV stage)  [tok_p, kt, c]
    v_dram = nc.dram_tensor("v_stage", [P, KT_N, NKVC * HD], f32r).ap()

    with tile.TileContext(nc) as tc:
        with tc.tile_pool(name="persist", bufs=1) as persist:
            # ---- constants ----
            rotm = persist.tile([P, P], f32r)
            ones128 = persist.tile([P, P], f32r)
            ones_col = persist.tile([P, 1], f32r)
            ones_row = persist.tile([1, P], f32r)
            eps_col = persist.tile([P, 1], f32)
            with tc.tile_pool(name="cscratch", bufs=1) as csp:
                rot_f = csp.tile([P, P], f32)
                nc.gpsimd.memset(rot_f, 0.0)
                # +1 where col = row + 64 (out[d'] = x[d'-64] for d' >= 64)
                nc.gpsimd.affine_select(
                    out=rot_f, in_=rot_f, compare_op=ALU.not_equal,
                    fill=1.0, base=64, pattern=[[-1, P]], channel_multiplier=1)
                # -1 where col = row - 64 (out[d'] = -x[d'+64] for d' < 64)
                nc.gpsimd.affine_select(
                    out=rot_f, in_=rot_f, compare_op=ALU.not_equal,
                    fill=-1.0, base=-64, pattern=[[-1, P]],
                    channel_multiplier=1)
                nc.vector.tensor_copy(rotm, rot_f)

                ones_f = csp.tile([P, P], f32)
                nc.vector.memset(ones_f, 1.0)
                nc.vector.tensor_copy(ones128, ones_f)
                nc.vector.tensor_copy(ones_col, ones_f[:, 0:1])
                nc.vector.tensor_copy(ones_row, ones_f[0:1, :])
            nc.vector.memset(eps_col, EPS)

            qn_col = persist.tile([P, 1], f32r)
            nc.sync.dma_start(out=qn_col, in_=qnw)
            kn_col = persist.tile([P, 1], f32r)
            nc.sync.dma_start(out=kn_col, in_=knw)

            # ---- persistent activations ----
            QT = persist.tile([P, NHC, QL], f32r)    # Q'^T  [d, head, q]
            KTt = persist.tile([P, NKVC, KL], f32r)  # K'^T  [d, kvh, k]

            def norm_rope(proj_ps, w_col, cosc, sinc, dst, midp, psums):
                """proj_ps [128,512] PSUM -> dst (f32r SBUF): rmsnorm+rope."""
                ssqp, rotp = psums
                cp = midp.tile([P, 512], f32r, tag="cp")
                nc.scalar.activation(cp, proj_ps, func=AF.Copy)
                sq = midp.tile([P, 512], f32r, tag="sq")
                nc.vector.tensor_mul(sq, cp, cp)
                ssq = ssqp.tile([P, 512], f32, tag="ssq")
                nc.tensor.matmul(ssq, ones128, sq, start=True, stop=True)
                lnv = midp.tile([P, 512], f32, tag="lnv")
                nc.scalar.activation(lnv, ssq, func=AF.Ln, scale=1.0 / HD,
                                     bias=eps_col)
                rstd = midp.tile([P, 512], f32, tag="rstd")
                nc.scalar.activation(rstd, lnv, func=AF.Exp, scale=-0.5)
                qn = midp.tile([P, 512], f32r, tag="cp")
                nc.vector.scalar_tensor_tensor(
                    qn, cp, w_col, rstd,
                    op0=ALU.mult, op1=ALU.mult)
                rot = rotp.tile([P, 512], f32, tag="rot")
                nc.tensor.matmul(rot, rotm, qn, start=True, stop=True)
                t1 = midp.tile([P, 512], f32, tag="sq")
                nc.vector.tensor_mul(t1, qn.bitcast(f32), cosc.bitcast(f32))
                t2 = midp.tile([P, 512], f32, tag="lnv")
                nc.vector.tensor_mul(t2, rot, sinc.bitcast(f32))
                nc.vector.tensor_add(dst, t1, t2)

            # ========= Stage QKV =========
            with tc.tile_pool(name="kv_c", bufs=2) as kvp, \
                 tc.tile_pool(name="kv_cs", bufs=2) as csp2, \
                 tc.tile_pool(name="kv_w", bufs=1) as wp, \
                 tc.tile_pool(name="q_w", bufs=2) as wqp, \
                 tc.tile_pool(name="kv_mid", bufs=2) as midp, \
                 tc.tile_pool(name="kv_vo", bufs=2) as vop, \
                 tc.tile_pool(name="kv_pk", bufs=2, space="PSUM") as pkp, \
                 tc.tile_pool(name="kv_pv", bufs=2, space="PSUM") as pvp, \
                 tc.tile_pool(name="kv_ssq", bufs=2, space="PSUM") as ssqp, \
                 tc.tile_pool(name="kv_rot", bufs=2, space="PSUM") as rotp:
                wk_sb = wp.tile([P, HT, NKVC * HD], f32r)
                nc.sync.dma_start(out=wk_sb, in_=wk)
                wv_sb = wp.tile([P, HT, NKVC * HD], f32r)
                nc.sync.dma_start(out=wv_sb, in_=wv)
                for ch in range(8):
                    sl = slice(ch * 512, (ch + 1) * 512)
                    kvc = kvp.tile([P, HT, 512], f32r, tag="kv")
                    nc.sync.dma_start(out=kvc, in_=kvT[:, :, sl])
                    cosc = csp2.tile([P, 512], f32r, tag="cos")
                    nc.sync.dma_start(out=cosc, in_=cosT[:, sl])
                    sinc = csp2.tile([P, 512], f32r, tag="sin")
                    nc.sync.dma_start(out=sinc, in_=sinT[:, sl])
                    # K^T projection + norm + rope (per kv head)
                    for kh in range(NKVC):
                        ps = pkp.tile([P, 512], f32, tag="proj")
                        for ht in range(HT):
                            nc.tensor.matmul(
                                ps, wk_sb[:, ht, kh * HD:(kh + 1) * HD],
                                kvc[:, ht, :],
                                start=(ht == 0), stop=(ht == HT - 1))
                        norm_rope(ps, kn_col, cosc, sinc,
                                  KTt[:, kh, sl], midp, (ssqp, rotp))
                    # V projection (natural layout), staged to HBM
                    for half in range(2):
                        psv = pvp.tile([P, 2, NKVC * HD], f32, tag="pv")
                        for tt2 in range(2):
                            tt = half * 2 + tt2
                            for ht in range(HT):
                                nc.tensor.matmul(
                                    psv[:, tt2, :],
                                    kvc[:, ht, tt * P:(tt + 1) * P],
                                    wv_sb[:, ht, :],
                                    start=(ht == 0), stop=(ht == HT - 1))
                        v_sb = vop.tile([P, 2, NKVC * HD], f32r, tag="vsb")
                        nc.scalar.activation(v_sb, psv, func=AF.Copy)
                        kt0 = ch * 4 + half * 2
                        nc.sync.dma_start(out=v_dram[:, kt0:kt0 + 2, :],
                                          in_=v_sb)
                    # Q projection for the noise rows (chunks 6, 7)
                    if ch >= 6:
                        qc = ch - 6
                        for qh in range(NHC):
                            wqt = wqp.tile([P, HT, HD], f32r, tag="wq")
                            nc.sync.dma_start(out=wqt, in_=wq[:, qh])
                            psq = pkp.tile([P, 512], f32, tag="proj")
                            for ht in range(HT):
                                nc.tensor.matmul(
                                    psq, wqt[:, ht, :], kvc[:, ht, :],
                                    start=(ht == 0), stop=(ht == HT - 1))
                            norm_rope(psq, qn_col, cosc, sinc,
                                      QT[:, qh, qc * 512:(qc + 1) * 512],
                                      midp, (ssqp, rotp))

            # ================= Stage ATT =================
            with tc.tile_pool(name="ot_persist", bufs=1) as otpp, \
                 tc.tile_pool(name="o_w0", bufs=1) as wopA:
                OT = otpp.tile([P, NHC, QL], f32r)
                w0 = wopA.tile([P, NHC, 1024], f32r)
                nc.sync.dma_start(out=w0, in_=wo[:, 0])
                _stage_att(nc, tc, OT, KTt, QT, v_dram, ones_col)
                _stage_o(nc, tc, OT, wo, out, w0)

    nc.compile()
    return nc


def _stage_att(nc, tc, OT, KTt, QT, v_dram, ones_col):
    with tc.tile_pool(name="at_v", bufs=2) as vp, \
         tc.tile_pool(name="at_et", bufs=3) as etp, \
         tc.tile_pool(name="at_or", bufs=2) as orp, \
         tc.tile_pool(name="at_rec", bufs=2) as recp, \
         tc.tile_pool(name="at_bc", bufs=2) as bcp, \
         tc.tile_pool(name="at_st", bufs=2, space="PSUM") as sTp, \
         tc.tile_pool(name="at_ops", bufs=1, space="PSUM") as oTp, \
         tc.tile_pool(name="at_den", bufs=2, space="PSUM") as denp:
        v_kv = None

        def normalize(lh, oT, dens):
            """Release oT PSUM via ACT copy; 1/den; gpsimd broadcast; DVE
            multiply. Zero PE involvement, all on idle ATT engines."""
            oraw = orp.tile([P, QL], f32r, tag="oraw", name=f"oraw_{lh}")
            nc.scalar.activation(oraw, oT, func=AF.Copy)
            rec = recp.tile([1, QL], f32, tag="rec", name=f"rec_{lh}")
            nc.vector.reciprocal(rec[:, 0:512], dens[0])
            nc.vector.reciprocal(rec[:, 512:1024], dens[1])
            bc = bcp.tile([P, QL], f32, tag="bc", name=f"bc_{lh}")
            nc.gpsimd.partition_broadcast(bc, rec)
            nc.vector.tensor_mul(OT[:, lh, :], oraw, bc.bitcast(f32r))

        pend = None  # (eT, dens, oT, kt, v_kv, lh) awaiting den/PV
        for lh in range(NHC):
            kvh = lh // 4
            if lh % 4 == 0:
                v_kv = vp.tile([P, KT_N, HD], f32r, tag="vkv")
                nc.sync.dma_start(
                    out=v_kv, in_=v_dram[:, :, kvh * HD:(kvh + 1) * HD])
            oT = oTp.tile([P, QL], f32, tag="oT")
            dens = [denp.tile([1, 512], f32, tag="den",
                              name=f"den_{lh}_{q}") for q in range(2)]
            for kt in range(KT_N):
                # S^T + exp for this kt (one FD-1024 exp over both q-halves)
                sT = sTp.tile([P, QL], f32, tag="sT")
                for qc in range(2):
                    nc.tensor.matmul(
                        sT[:, qc * 512:(qc + 1) * 512],
                        KTt[:, kvh, kt * P:(kt + 1) * P],
                        QT[:, lh, qc * 512:(qc + 1) * 512],
                        start=True, stop=True)
                eT = etp.tile([P, QL], f32r, tag="eT")
                nc.scalar.activation(eT, sT, func=AF.Exp, scale=SCALE)
                # den/PV for the PREVIOUS kt (software pipeline: PE never
                # waits on the exp it just launched)
                if pend is not None:
                    peT, pdens, poT, pkt, pv, plh = pend
                    for qc in range(2):
                        nc.tensor.matmul(
                            pdens[qc], ones_col,
                            peT[:, qc * 512:(qc + 1) * 512],
                            start=(pkt == 0), stop=(pkt == KT_N - 1))
                        nc.tensor.matmul(
                            poT[:, qc * 512:(qc + 1) * 512],
                            pv[:, pkt, :],
                            peT[:, qc * 512:(qc + 1) * 512],
                            start=(pkt == 0), stop=(pkt == KT_N - 1))
                    if pkt == KT_N - 1:
                        normalize(plh, poT, pdens)
                pend = (eT, dens, oT, kt, v_kv, lh)
        # drain the last head
        peT, pdens, poT, pkt, pv, plh = pend
        for qc in range(2):
            nc.tensor.matmul(pdens[qc], ones_col,
                             peT[:, qc * 512:(qc + 1) * 512],
                             start=(pkt == 0), stop=(pkt == KT_N - 1))
            nc.tensor.matmul(poT[:, qc * 512:(qc + 1) * 512],
                             pv[:, pkt, :],
                             peT[:, qc * 512:(qc + 1) * 512],
                             start=(pkt == 0), stop=(pkt == KT_N - 1))
        normalize(plh, poT, pdens)


def _stage_o(nc, tc, OT, wo, out, w0):
    # Two column passes: pass 0 uses pre-loaded w0 (cols 0:1024); pass 1's
    # w1 load overlaps pass 0's matmuls.
    with tc.tile_pool(name="o_w1", bufs=1) as wopB, \
         tc.tile_pool(name="o_out", bufs=3) as outp, \
         tc.tile_pool(name="o_ps", bufs=4, space="PSUM") as opsp:
        w1 = wopB.tile([P, NHC, 1024], f32r)
        nc.sync.dma_start(out=w1, in_=wo[:, 1])
        for half, wsb in ((0, w0), (1, w1)):
            for qt in range(8):
                ob = outp.tile([P, 1024], f32, tag="ob")
                pss = [opsp.tile([P, 512], f32, tag="ops",
                                 name=f"ops_{half}_{qt}_{i}") for i in range(2)]
                for ci in range(NHC):
                    for nch in range(2):
                        nc.tensor.matmul(
                            pss[nch], OT[:, ci, qt * P:(qt + 1) * P],
                            wsb[:, ci, nch * 512:(nch + 1) * 512],
                            start=(ci == 0), stop=(ci == NHC - 1))
                for nch in range(2):
                    nc.any.tensor_copy(ob[:, nch * 512:(nch + 1) * 512],
                                       pss[nch])
                nc.sync.dma_start(
                    out=out[qt * P:(qt + 1) * P,
                            half * 1024:(half + 1) * 1024],
                    in_=ob)


def _get_nc():
    global _NC
    if _NC is None:
        _NC = build_nc()
    return _NC


def _make_in_maps(noise, ctx, cos, sin, Wq, Wk, Wv, Wo, qn_w, kn_w):
    noise = np.asarray(noise, np.float32)
    ctx = np.asarray(ctx, np.float32)
    cos = np.asarray(cos, np.float32)
    sin = np.asarray(sin, np.float32)
    Wq = np.asarray(Wq, np.float32)
    Wk = np.asarray(Wk, np.float32)
    Wv = np.asarray(Wv, np.float32)
    Wo = np.asarray(Wo, np.float32)
    qn_w = np.asarray(qn_w, np.float32).reshape(P, 1)
    kn_w = np.asarray(kn_w, np.float32).reshape(P, 1)
    B = noise.shape[0]
    in_maps = []
    for b in range(B):
        kv_b = np.concatenate([ctx[b], noise[b]], axis=0)      # [KL, H]
        # [p, ht, t]
        kvT_b = np.ascontiguousarray(
            kv_b.T.reshape(HT, P, KL).transpose(1, 0, 2))
        cosT_b = np.ascontiguousarray(cos[b].T)                 # [P, KL]
        sinT_b = np.ascontiguousarray(sin[b].T)
        for g in range(2):
            Wq_g = Wq[:, g * 1024:(g + 1) * 1024]
            Wk_g = Wk[:, g * 256:(g + 1) * 256]
            Wv_g = Wv[:, g * 256:(g + 1) * 256]
            Wo_g = Wo[g * 1024:(g + 1) * 1024, :]
            in_maps.append({
                "kvT": kvT_b,
                "cosT": cosT_b,
                "sinT": sinT_b,
                "wq": np.ascontiguousarray(
                    Wq_g.reshape(HT, P, NHC, HD).transpose(1, 2, 0, 3)),
                "wk": np.ascontiguousarray(
                    Wk_g.reshape(HT, P, NKVC * HD).transpose(1, 0, 2)),
                "wv": np.ascontiguousarray(
                    Wv_g.reshape(HT, P, NKVC * HD).transpose(1, 0, 2)),
                "wo": np.ascontiguousarray(
                    Wo_g.reshape(NHC, P, 2, 1024).transpose(1, 2, 0, 3)),
                "qnw": qn_w,
                "knw": kn_w,
            })
    return in_maps


def _install_profile_hook():
    """Provide antenv.axon_hooks (absent in this container) so
    run_bass_kernel_spmd(trace=True) can NTFF-profile via libaxon_pjrt."""
    import types
    if "antenv.axon_hooks" not in sys.modules:
        import antenv
        mod = types.ModuleType("antenv.axon_hooks")
        _state = {}
        mod.set_axon_ntff_profile_hook = lambda h: _state.__setitem__("h", h)
        mod.get_axon_ntff_profile_hook = lambda: _state.get("h")
        sys.modules["antenv.axon_hooks"] = mod
        antenv.axon_hooks = mod
        from trn_agent_boot.trn_boot import _ntff_profile_via_ctypes
        mod.set_axon_ntff_profile_hook(
            _ntff_profile_via_ctypes("/opt/axon/libaxon_pjrt.so"))
    import concourse.bass_utils as bu
    bu.upload_artifacts = lambda tmpdir: tmpdir


def run(inputs, trace=False, tmpdir=None):
    """Run on 8 cores; returns (output [4,1024,2048], results)."""
    nc = _get_nc()
    in_maps = _make_in_maps(**inputs)
    if trace:
        _install_profile_hook()
    res = run_bass_kernel_spmd(nc, in_maps, core_ids=list(range(8)),
                               trace=trace, tmpdir=tmpdir,
                               trace_cores=[0] if trace else None)
    outs = [res.results[i]["out"] for i in range(8)]
    full = np.stack([outs[2 * b] + outs[2 * b + 1] for b in range(4)], axis=0)
    return full.astype(np.float32), res


def kernel(**inputs):
    out, _ = run(inputs, trace=False)
    return out


def _attr(i, name):
    v = getattr(i, name, None)
    try:
        if callable(v):
            v = v()
    except Exception:
        v = "?"
    return v


def summarize_trace(res, top=30):
    """Per-engine busy time + per-op totals."""
    if not res.instructions_and_trace:
        print("no trace")
        return
    insts, trace_path = res.instructions_and_trace
    from collections import defaultdict
    eng_busy = defaultdict(int)
    eng_n = defaultdict(int)
    op_cost = defaultdict(int)
    op_n = defaultdict(int)
    t0 = min(i.timestamp for i in insts)
    t1 = max(i.end_timestamp for i in insts)
    for i in insts:
        eng = str(_attr(i, "engine"))
        op = str(_attr(i, "op_name"))
        eng_busy[eng] += i.duration
        eng_n[eng] += 1
        op_cost[(eng, op)] += i.duration
        op_n[(eng, op)] += 1
    span = t1 - t0
    print(f"trace: {trace_path}")
    print(f"span: {span} ns")
    for e in sorted(eng_busy, key=lambda e: -eng_busy[e]):
        print(f"  {e:14s} busy {eng_busy[e]:>10} ns "
              f"({100.0*eng_busy[e]/span:5.1f}%)  n={eng_n[e]}")
    print("per (engine, op): total_ns, n, avg_ns")
    for (e, op), c in sorted(op_cost.items(), key=lambda kv: -kv[1])[:top]:
        n = op_n[(e, op)]
        print(f"  {c:>10} ns  n={n:<5d} avg={c//max(n,1):>7} ns  {e:12s} {op}")


# revision 7
# speedup vs baseline: 1.4965x; 1.4965x over previous
"""DFlashAttention Trainium2 kernel (8 NeuronCores, SPMD, no collectives).

Problem (hardcoded shapes): B=4, QL=1024, CL=3072, KL=4096, H=2048,
NH=16 q-heads, NKV=4 kv-heads, HD=128.

Sharding: core i = (batch b = i//2, head-group g = i%2). Each core computes
8 q-heads / 2 kv-heads for one batch and produces a partial o_proj output
(contraction over its head block of Wo); the host sums the two partials per
batch.

v2 design (vs v1 baseline at 1106us):
  - ALL transposes happen on the HOST: kv^T, cos^T, sin^T and all weights
    are pre-arranged into their exact SBUF layouts, so the PE never runs a
    transpose (was 576 transposes + 576 PSUM->SBUF copies).
  - RMSNorm without PE broadcast or DVE reciprocal: ssq matmul uses a
    [128,128] ones stationary so the partition-sum lands broadcast across
    all 128 partitions; rstd = exp(-0.5*ln(var+eps)) on ACT (Square, Ln,
    Exp and Copy all live in the single natural_log_exp_and_others table
    set - zero table switches in the whole kernel).
  - norm apply fused: qn = (proj * w[d]) * rstd via one scalar_tensor_tensor.
  - ATT: sT tiles are [128,1024] (2 PSUM banks) so exp runs once per
    (head, kt) at FD=1024 (1147ns) under 6 FD-512 matmuls (~1280ns) - the
    ACT engine stays just under the PE.
  - softmax normalize off the critical path: oT PSUM is released by an ACT
    copy; 1/den via DVE reciprocal on [1,512] tiles; broadcast via gpsimd
    partition_broadcast; final multiply on DVE. No PE participation.
"""
import os
import sys

sys.path.insert(0, "/opt/trn_rl_repo")

import numpy as np

import concourse.bass as bass
import concourse.tile as tile
from concourse import bacc, mybir
from concourse.bass_utils import run_bass_kernel_spmd

f32 = mybir.dt.float32
f32r = mybir.dt.float32r
AF = mybir.ActivationFunctionType
ALU = mybir.AluOpType

P = 128
H = 2048
HT = H // P          # 16 h-tiles
QL = 1024
CL = 3072
KL = CL + QL         # 4096
KT_N = KL // P       # 32 k-tiles
HD = 128
NHC = 8              # q heads per core
NKVC = 2             # kv heads per core
SCALE = HD ** -0.5
EPS = 1e-6

_NC = None


def build_nc():
    nc = bacc.Bacc("TRN2", target_bir_lowering=False, debug=False)

    # host-pre-arranged layouts (see _make_in_maps)
    kvT = nc.dram_tensor("kvT", [P, HT, KL], f32r, kind="ExternalInput").ap()
    cosT = nc.dram_tensor("cosT", [P, KL], f32r, kind="ExternalInput").ap()
    sinT = nc.dram_tensor("sinT", [P, KL], f32r, kind="ExternalInput").ap()
    wq = nc.dram_tensor("wq", [P, NHC, HT, HD], f32r, kind="ExternalInput").ap()
    wk = nc.dram_tensor("wk", [P, HT, NKVC * HD], f32r, kind="ExternalInput").ap()
    wv = nc.dram_tensor("wv", [P, HT, NKVC * HD], f32r, kind="ExternalInput").ap()
    wo = nc.dram_tensor("wo", [P, 2, NHC, 1024], f32r, kind="ExternalInput").ap()
    qnw = nc.dram_tensor("qnw", [P, 1], f32r, kind="ExternalInput").ap()
    knw = nc.dram_tensor("knw", [P, 1], f32r, kind="ExternalInput").ap()
    out = nc.dram_tensor("out", [QL, H], f32, kind="ExternalOutput").ap()

    # HBM staging for V (saves SBUF during the KV stage)  [tok_p, kt, c]
    v_dram = nc.dram_tensor("v_stage", [P, KT_N, NKVC * HD], f32r).ap()

    with tile.TileContext(nc) as tc:
        with tc.tile_pool(name="persist", bufs=1) as persist:
            # ---- constants ----
            rotm = persist.tile([P, P], f32r)
            ones128 = persist.tile([P, P], f32r)
            ones_col = persist.tile([P, 1], f32r)
            ones_row = persist.tile([1, P], f32r)
            eps_col = persist.tile([P, 1], f32)
            with tc.tile_pool(name="cscratch", bufs=1) as csp:
                rot_f = csp.tile([P, P], f32)
                nc.gpsimd.memset(rot_f, 0.0)
                # +1 where col = row + 64 (out[d'] = x[d'-64] for d' >= 64)
                nc.gpsimd.affine_select(
                    out=rot_f, in_=rot_f, compare_op=ALU.not_equal,
                    fill=1.0, base=64, pattern=[[-1, P]], channel_multiplier=1)
                # -1 where col = row - 64 (out[d'] = -x[d'+64] for d' < 64)
                nc.gpsimd.affine_select(
                    out=rot_f, in_=rot_f, compare_op=ALU.not_equal,
                    fill=-1.0, base=-64, pattern=[[-1, P]],
                    channel_multiplier=1)
                nc.vector.tensor_copy(rotm, rot_f)

                ones_f = csp.tile([P, P], f32)
                nc.vector.memset(ones_f, 1.0)
                nc.vector.tensor_copy(ones128, ones_f)
                nc.vector.tensor_copy(ones_col, ones_f[:, 0:1])
                nc.vector.tensor_copy(ones_row, ones_f[0:1, :])
            nc.vector.memset(eps_col, EPS)

            qn_col = persist.tile([P, 1], f32r)
            nc.sync.dma_start(out=qn_col, in_=qnw)
            kn_col = persist.tile([P, 1], f32r)
            nc.sync.dma_start(out=kn_col, in_=knw)

            # ---- persistent activations ----
            QT = persist.tile([P, NHC, QL], f32r)    # Q'^T  [d, head, q]
            KTt = persist.tile([P, NKVC, KL], f32r)  # K'^T  [d, kvh, k]

            def norm_rope(proj_ps, w_col, cosc, sinc, dst, midp, psums):
                """proj_ps [128,512] PSUM -> dst (f32r SBUF): rmsnorm+rope."""
                ssqp, rotp = psums
                sq = midp.tile([P, 512], f32r, tag="sq")
                nc.scalar.activation(sq, proj_ps, func=AF.Square)
                ssq = ssqp.tile([P, 512], f32, tag="ssq")
                nc.tensor.matmul(ssq, ones128, sq, start=True, stop=True)
                # rstd = 1/sqrt(var+eps); Square/AbsRsqrt/Copy share one
                # table set, so the whole kernel does 2 ACT_TABLE_LOADs.
                rstd = midp.tile([P, 512], f32, tag="rstd")
                nc.scalar.activation(rstd, ssq, func=AF.Abs_reciprocal_sqrt,
                                     scale=1.0 / HD, bias=eps_col)
                qn = midp.tile([P, 512], f32r, tag="qn")
                nc.vector.scalar_tensor_tensor(
                    qn, proj_ps, w_col, rstd,
                    op0=ALU.mult, op1=ALU.mult)
                rot = rotp.tile([P, 512], f32, tag="rot")
                nc.tensor.matmul(rot, rotm, qn, start=True, stop=True)
                t1 = midp.tile([P, 512], f32, tag="sq")
                nc.vector.tensor_mul(t1, qn.bitcast(f32), cosc.bitcast(f32))
                t2 = midp.tile([P, 512], f32, tag="rstd")
                nc.vector.tensor_mul(t2, rot, sinc.bitcast(f32))
                nc.vector.tensor_add(dst, t1, t2)

            # ========= Stage QKV =========
            with tc.tile_pool(name="kv_c", bufs=2) as kvp, \
                 tc.tile_pool(name="kv_cs", bufs=2) as csp2, \
                 tc.tile_pool(name="kv_w", bufs=1) as wp, \
                 tc.tile_pool(name="q_w", bufs=2) as wqp, \
                 tc.tile_pool(name="kv_mid", bufs=2) as midp, \
                 tc.tile_pool(name="kv_vo", bufs=2) as vop, \
                 tc.tile_pool(name="kv_pk", bufs=2, space="PSUM") as pkp, \
                 tc.tile_pool(name="kv_pv", bufs=2, space="PSUM") as pvp, \
                 tc.tile_pool(name="kv_ssq", bufs=2, space="PSUM") as ssqp, \
                 tc.tile_pool(name="kv_rot", bufs=2, space="PSUM") as rotp:
                wk_sb = wp.tile([P, HT, NKVC * HD], f32r)
                nc.sync.dma_start(out=wk_sb, in_=wk)
                wv_sb = wp.tile([P, HT, NKVC * HD], f32r)
                nc.sync.dma_start(out=wv_sb, in_=wv)
                for ch in range(8):
                    sl = slice(ch * 512, (ch + 1) * 512)
                    kvc = kvp.tile([P, HT, 512], f32r, tag="kv")
                    nc.sync.dma_start(out=kvc, in_=kvT[:, :, sl])
                    cosc = csp2.tile([P, 512], f32r, tag="cos")
                    nc.sync.dma_start(out=cosc, in_=cosT[:, sl])
                    sinc = csp2.tile([P, 512], f32r, tag="sin")
                    nc.sync.dma_start(out=sinc, in_=sinT[:, sl])
                    # K^T projection + norm + rope (per kv head)
                    for kh in range(NKVC):
                        ps = pkp.tile([P, 512], f32, tag="proj")
                        for ht in range(HT):
                            nc.tensor.matmul(
                                ps, wk_sb[:, ht, kh * HD:(kh + 1) * HD],
                                kvc[:, ht, :],
                                start=(ht == 0), stop=(ht == HT - 1))
                        norm_rope(ps, kn_col, cosc, sinc,
                                  KTt[:, kh, sl], midp, (ssqp, rotp))
                    # V projection (natural layout), staged to HBM
                    for half in range(2):
                        psv = pvp.tile([P, 2, NKVC * HD], f32, tag="pv")
                        for tt2 in range(2):
                            tt = half * 2 + tt2
                            for ht in range(HT):
                                nc.tensor.matmul(
                                    psv[:, tt2, :],
                                    kvc[:, ht, tt * P:(tt + 1) * P],
                                    wv_sb[:, ht, :],
                                    start=(ht == 0), stop=(ht == HT - 1))
                        v_sb = vop.tile([P, 2, NKVC * HD], f32r, tag="vsb")
                        nc.scalar.activation(v_sb, psv, func=AF.Copy)
                        kt0 = ch * 4 + half * 2
                        nc.sync.dma_start(out=v_dram[:, kt0:kt0 + 2, :],
                                          in_=v_sb)
                    # Q projection for the noise rows (chunks 6, 7)
                    if ch >= 6:
                        qc = ch - 6
                        for qh in range(NHC):
                            wqt = wqp.tile([P, HT, HD], f32r, tag="wq")
                            nc.sync.dma_start(out=wqt, in_=wq[:, qh])
                            psq = pkp.tile([P, 512], f32, tag="proj")
                            for ht in range(HT):
                                nc.tensor.matmul(
                                    psq, wqt[:, ht, :], kvc[:, ht, :],
                                    start=(ht == 0), stop=(ht == HT - 1))
                            norm_rope(psq, qn_col, cosc, sinc,
                                      QT[:, qh, qc * 512:(qc + 1) * 512],
                                      midp, (ssqp, rotp))

            # ================= Stage ATT =================
            with tc.tile_pool(name="ot_persist", bufs=1) as otpp, \
                 tc.tile_pool(name="o_w0", bufs=1) as wopA:
                OT = otpp.tile([P, NHC, QL], f32r)
                w0 = wopA.tile([P, NHC, 1024], f32r)
                nc.sync.dma_start(out=w0, in_=wo[:, 0])
                _stage_att(nc, tc, OT, KTt, QT, v_dram, ones_col)
                _stage_o(nc, tc, OT, wo, out, w0)

    nc.compile()
    return nc


def _stage_att(nc, tc, OT, KTt, QT, v_dram, ones_col):
    with tc.tile_pool(name="at_v", bufs=2) as vp, \
         tc.tile_pool(name="at_et", bufs=3) as etp, \
         tc.tile_pool(name="at_or", bufs=2) as orp, \
         tc.tile_pool(name="at_rec", bufs=2) as recp, \
         tc.tile_pool(name="at_bc", bufs=2) as bcp, \
         tc.tile_pool(name="at_st", bufs=2, space="PSUM") as sTp, \
         tc.tile_pool(name="at_ops", bufs=1, space="PSUM") as oTp, \
         tc.tile_pool(name="at_den", bufs=2, space="PSUM") as denp:
        v_kv = None

        def normalize(lh, oT, dens):
            """Release oT PSUM via ACT copy; 1/den; gpsimd broadcast; DVE
            multiply. Zero PE involvement, all on idle ATT engines."""
            oraw = orp.tile([P, QL], f32r, tag="oraw", name=f"oraw_{lh}")
            nc.scalar.activation(oraw, oT, func=AF.Copy)
            rec = recp.tile([1, QL], f32, tag="rec", name=f"rec_{lh}")
            nc.vector.reciprocal(rec[:, 0:512], dens[0])
            nc.vector.reciprocal(rec[:, 512:1024], dens[1])
            bc = bcp.tile([P, QL], f32, tag="bc", name=f"bc_{lh}")
            nc.gpsimd.partition_broadcast(bc, rec)
            nc.vector.tensor_mul(OT[:, lh, :], oraw, bc.bitcast(f32r))

        pend = None  # (eT, dens, oT, kt, v_kv, lh) awaiting den/PV
        for lh in range(NHC):
            kvh = lh // 4
            if lh % 4 == 0:
                v_kv = vp.tile([P, KT_N, HD], f32r, tag="vkv")
                nc.sync.dma_start(
                    out=v_kv, in_=v_dram[:, :, kvh * HD:(kvh + 1) * HD])
            oT = oTp.tile([P, QL], f32, tag="oT")
            dens = [denp.tile([1, 512], f32, tag="den",
                              name=f"den_{lh}_{q}") for q in range(2)]
            for kt in range(KT_N):
                # S^T + exp for this kt (one FD-1024 exp over both q-halves)
                sT = sTp.tile([P, QL], f32, tag="sT")
                for qc in range(2):
                    nc.tensor.matmul(
                        sT[:, qc * 512:(qc + 1) * 512],
                        KTt[:, kvh, kt * P:(kt + 1) * P],
                        QT[:, lh, qc * 512:(qc + 1) * 512],
                        start=True, stop=True)
                eT = etp.tile([P, QL], f32r, tag="eT")
                nc.scalar.activation(eT, sT, func=AF.Exp, scale=SCALE)
                # den/PV for the PREVIOUS kt (software pipeline: PE never
                # waits on the exp it just launched)
                if pend is not None:
                    peT, pdens, poT, pkt, pv, plh = pend
                    for qc in range(2):
                        nc.tensor.matmul(
                            pdens[qc], ones_col,
                            peT[:, qc * 512:(qc + 1) * 512],
                            start=(pkt == 0), stop=(pkt == KT_N - 1))
                        nc.tensor.matmul(
                            poT[:, qc * 512:(qc + 1) * 512],
                            pv[:, pkt, :],
                            peT[:, qc * 512:(qc + 1) * 512],
                            start=(pkt == 0), stop=(pkt == KT_N - 1))
                    if pkt == KT_N - 1:
                        normalize(plh, poT, pdens)
                pend = (eT, dens, oT, kt, v_kv, lh)
        # drain the last head
        peT, pdens, poT, pkt, pv, plh = pend
        for qc in range(2):
            nc.tensor.matmul(pdens[qc], ones_col,
                             peT[:, qc * 512:(qc + 1) * 512],
                             start=(pkt == 0), stop=(pkt == KT_N - 1))
            nc.tensor.matmul(poT[:, qc * 512:(qc + 1) * 512],
                             pv[:, pkt, :],
                             peT[:, qc * 512:(qc + 1) * 512],
                             start=(pkt == 0), stop=(pkt == KT_N - 1))
        normalize(plh, poT, pdens)


def _stage_o(nc, tc, OT, wo, out, w0):
    # Two column passes: pass 0 uses pre-loaded w0 (cols 0:1024); pass 1's
    # w1 load overlaps pass 0's matmuls.
    with tc.tile_pool(name="o_w1", bufs=1) as wopB, \
         tc.tile_pool(name="o_out", bufs=3) as outp, \
         tc.tile_pool(name="o_ps", bufs=4, space="PSUM") as opsp:
        w1 = wopB.tile([P, NHC, 1024], f32r)
        nc.sync.dma_start(out=w1, in_=wo[:, 1])
        for half, wsb in ((0, w0), (1, w1)):
            for qt in range(8):
                ob = outp.tile([P, 1024], f32, tag="ob")
                pss = [opsp.tile([P, 512], f32, tag="ops",
                                 name=f"ops_{half}_{qt}_{i}") for i in range(2)]
                for ci in range(NHC):
                    for nch in range(2):
                        nc.tensor.matmul(
                            pss[nch], OT[:, ci, qt * P:(qt + 1) * P],
                            wsb[:, ci, nch * 512:(nch + 1) * 512],
                            start=(ci == 0), stop=(ci == NHC - 1))
                for nch in range(2):
                    nc.any.tensor_copy(ob[:, nch * 512:(nch + 1) * 512],
                                       pss[nch])
                nc.sync.dma_start(
                    out=out[qt * P:(qt + 1) * P,
                            half * 1024:(half + 1) * 1024],
                    in_=ob)


def _get_nc():
    global _NC
    if _NC is None:
        _NC = build_nc()
    return _NC


def _make_in_maps(noise, ctx, cos, sin, Wq, Wk, Wv, Wo, qn_w, kn_w):
    noise = np.asarray(noise, np.float32)
    ctx = np.asarray(ctx, np.float32)
    cos = np.asarray(cos, np.float32)
    sin = np.asarray(sin, np.float32)
    Wq = np.asarray(Wq, np.float32)
    Wk = np.asarray(Wk, np.float32)
    Wv = np.asarray(Wv, np.float32)
    Wo = np.asarray(Wo, np.float32)
    qn_w = np.asarray(qn_w, np.float32).reshape(P, 1)
    kn_w = np.asarray(kn_w, np.float32).reshape(P, 1)
    B = noise.shape[0]
    in_maps = []
    for b in range(B):
        kv_b = np.concatenate([ctx[b], noise[b]], axis=0)      # [KL, H]
        # [p, ht, t]
        kvT_b = np.ascontiguousarray(
            kv_b.T.reshape(HT, P, KL).transpose(1, 0, 2))
        cosT_b = np.ascontiguousarray(cos[b].T)                 # [P, KL]
        sinT_b = np.ascontiguousarray(sin[b].T)
        for g in range(2):
            Wq_g = Wq[:, g * 1024:(g + 1) * 1024]
            Wk_g = Wk[:, g * 256:(g + 1) * 256]
            Wv_g = Wv[:, g * 256:(g + 1) * 256]
            Wo_g = Wo[g * 1024:(g + 1) * 1024, :]
            in_maps.append({
                "kvT": kvT_b,
                "cosT": cosT_b,
                "sinT": sinT_b,
                "wq": np.ascontiguousarray(
                    Wq_g.reshape(HT, P, NHC, HD).transpose(1, 2, 0, 3)),
                "wk": np.ascontiguousarray(
                    Wk_g.reshape(HT, P, NKVC * HD).transpose(1, 0, 2)),
                "wv": np.ascontiguousarray(
                    Wv_g.reshape(HT, P, NKVC * HD).transpose(1, 0, 2)),
                "wo": np.ascontiguousarray(
                    Wo_g.reshape(NHC, P, 2, 1024).transpose(1, 2, 0, 3)),
                "qnw": qn_w,
                "knw": kn_w,
            })
    return in_maps


def _install_profile_hook():
    """Provide antenv.axon_hooks (absent in this container) so
    run_bass_kernel_spmd(trace=True) can NTFF-profile via libaxon_pjrt."""
    import types
    if "antenv.axon_hooks" not in sys.modules:
        import antenv
        mod = types.ModuleType("antenv.axon_hooks")
        _state = {}
        mod.set_axon_ntff_profile_hook = lambda h: _state.__setitem__("h", h)
        mod.get_axon_ntff_profile_hook = lambda: _state.get("h")
        sys.modules["antenv.axon_hooks"] = mod
        antenv.axon_hooks = mod
        from trn_agent_boot.trn_boot import _ntff_profile_via_ctypes
        mod.set_axon_ntff_profile_hook(
            _ntff_profile_via_ctypes("/opt/axon/libaxon_pjrt.so"))
    import concourse.bass_utils as bu
    bu.upload_artifacts = lambda tmpdir: tmpdir


def run(inputs, trace=False, tmpdir=None):
    """Run on 8 cores; returns (output [4,1024,2048], results)."""
    nc = _get_nc()
    in_maps = _make_in_maps(**inputs)
    if trace:
        _install_profile_hook()
    res = run_bass_kernel_spmd(nc, in_maps, core_ids=list(range(8)),
                               trace=trace, tmpdir=tmpdir,
                               trace_cores=[0] if trace else None)
    outs = [res.results[i]["out"] for i in range(8)]
    full = np.stack([outs[2 * b] + outs[2 * b + 1] for b in range(4)], axis=0)
    return full.astype(np.float32), res


def kernel(**inputs):
    out, _ = run(inputs, trace=False)
    return out


def _attr(i, name):
    v = getattr(i, name, None)
    try:
        if callable(v):
            v = v()
    except Exception:
        v = "?"
    return v


def summarize_trace(res, top=30):
    """Per-engine busy time + per-op totals."""
    if not res.instructions_and_trace:
        print("no trace")
        return
    insts, trace_path = res.instructions_and_trace
    from collections import defaultdict
    eng_busy = defaultdict(int)
    eng_n = defaultdict(int)
    op_cost = defaultdict(int)
    op_n = defaultdict(int)
    t0 = min(i.timestamp for i in insts)
    t1 = max(i.end_timestamp for i in insts)
    for i in insts:
        eng = str(_attr(i, "engine"))
        op = str(_attr(i, "op_name"))
        eng_busy[eng] += i.duration
        eng_n[eng] += 1
        op_cost[(eng, op)] += i.duration
        op_n[(eng, op)] += 1
    span = t1 - t0
    print(f"trace: {trace_path}")
    print(f"span: {span} ns")
    for e in sorted(eng_busy, key=lambda e: -eng_busy[e]):
        print(f"  {e:14s} busy {eng_busy[e]:>10} ns "
              f"({100.0*eng_busy[e]/span:5.1f}%)  n={eng_n[e]}")
    print("per (engine, op): total_ns, n, avg_ns")
    for (e, op), c in sorted(op_cost.items(), key=lambda kv: -kv[1])[:top]:
        n = op_n[(e, op)]
        print(f"  {c:>10} ns  n={n:<5d} avg={c//max(n,1):>7} ns  {e:12s} {op}")


# revision 10
# speedup vs baseline: 1.5611x; 1.0431x over previous
"""DFlashAttention Trainium2 kernel (8 NeuronCores, SPMD, no collectives).

Problem (hardcoded shapes): B=4, QL=1024, CL=3072, KL=4096, H=2048,
NH=16 q-heads, NKV=4 kv-heads, HD=128.

Sharding: core i = (batch b = i//2, head-group g = i%2). Each core computes
8 q-heads / 2 kv-heads for one batch and produces a partial o_proj output
(contraction over its head block of Wo); the host sums the two partials per
batch.

v2 design (vs v1 baseline at 1106us):
  - ALL transposes happen on the HOST: kv^T, cos^T, sin^T and all weights
    are pre-arranged into their exact SBUF layouts, so the PE never runs a
    transpose (was 576 transposes + 576 PSUM->SBUF copies).
  - RMSNorm without PE broadcast or DVE reciprocal: ssq matmul uses a
    [128,128] ones stationary so the partition-sum lands broadcast across
    all 128 partitions; rstd = exp(-0.5*ln(var+eps)) on ACT (Square, Ln,
    Exp and Copy all live in the single natural_log_exp_and_others table
    set - zero table switches in the whole kernel).
  - norm apply fused: qn = (proj * w[d]) * rstd via one scalar_tensor_tensor.
  - ATT: sT tiles are [128,1024] (2 PSUM banks) so exp runs once per
    (head, kt) at FD=1024 (1147ns) under 6 FD-512 matmuls (~1280ns) - the
    ACT engine stays just under the PE.
  - softmax normalize off the critical path: oT PSUM is released by an ACT
    copy; 1/den via DVE reciprocal on [1,512] tiles; broadcast via gpsimd
    partition_broadcast; final multiply on DVE. No PE participation.
"""
import os
import sys

sys.path.insert(0, "/opt/trn_rl_repo")

import numpy as np

import concourse.bass as bass
import concourse.tile as tile
from concourse import bacc, mybir
from concourse.bass_utils import run_bass_kernel_spmd

f32 = mybir.dt.float32
f32r = mybir.dt.float32r
AF = mybir.ActivationFunctionType
ALU = mybir.AluOpType

P = 128
H = 2048
HT = H // P          # 16 h-tiles
QL = 1024
CL = 3072
KL = CL + QL         # 4096
KT_N = KL // P       # 32 k-tiles
HD = 128
NHC = 8              # q heads per core
NKVC = 2             # kv heads per core
SCALE = HD ** -0.5
EPS = 1e-6

_NC = None


def build_nc():
    nc = bacc.Bacc("TRN2", target_bir_lowering=False, debug=False)

    # host-pre-arranged layouts (see _make_in_maps)
    kvT = nc.dram_tensor("kvT", [P, HT, KL], f32r, kind="ExternalInput").ap()
    cosT = nc.dram_tensor("cosT", [P, KL], f32r, kind="ExternalInput").ap()
    sinT = nc.dram_tensor("sinT", [P, KL], f32r, kind="ExternalInput").ap()
    wq = nc.dram_tensor("wq", [P, NHC, HT, HD], f32r, kind="ExternalInput").ap()
    wk = nc.dram_tensor("wk", [P, HT, NKVC * HD], f32r, kind="ExternalInput").ap()
    wv = nc.dram_tensor("wv", [P, HT, NKVC * HD], f32r, kind="ExternalInput").ap()
    wo = nc.dram_tensor("wo", [P, 2, NHC, 1024], f32r, kind="ExternalInput").ap()
    qnw = nc.dram_tensor("qnw", [P, 1], f32r, kind="ExternalInput").ap()
    knw = nc.dram_tensor("knw", [P, 1], f32r, kind="ExternalInput").ap()
    out = nc.dram_tensor("out", [QL, H], f32, kind="ExternalOutput").ap()

    # HBM staging for V (saves SBUF during the KV stage)  [tok_p, kt, c]
    v_dram = nc.dram_tensor("v_stage", [P, KT_N, NKVC * HD], f32r).ap()

    with tile.TileContext(nc) as tc:
        with tc.tile_pool(name="persist", bufs=1) as persist:
            # ---- constants ----
            rotm = persist.tile([P, P], f32r)
            ones128 = persist.tile([P, P], f32r)
            ones_col = persist.tile([P, 1], f32r)
            ones_row = persist.tile([1, P], f32r)
            eps_col = persist.tile([P, 1], f32)
            with tc.tile_pool(name="cscratch", bufs=1) as csp:
                rot_f = csp.tile([P, P], f32)
                nc.gpsimd.memset(rot_f, 0.0)
                # +1 where col = row + 64 (out[d'] = x[d'-64] for d' >= 64)
                nc.gpsimd.affine_select(
                    out=rot_f, in_=rot_f, compare_op=ALU.not_equal,
                    fill=1.0, base=64, pattern=[[-1, P]], channel_multiplier=1)
                # -1 where col = row - 64 (out[d'] = -x[d'+64] for d' < 64)
                nc.gpsimd.affine_select(
                    out=rot_f, in_=rot_f, compare_op=ALU.not_equal,
                    fill=-1.0, base=-64, pattern=[[-1, P]],
                    channel_multiplier=1)
                nc.vector.tensor_copy(rotm, rot_f)

                ones_f = csp.tile([P, P], f32)
                nc.vector.memset(ones_f, 1.0)
                nc.vector.tensor_copy(ones128, ones_f)
                nc.vector.tensor_copy(ones_col, ones_f[:, 0:1])
                nc.vector.tensor_copy(ones_row, ones_f[0:1, :])
            nc.vector.memset(eps_col, EPS)

            qn_col = persist.tile([P, 1], f32r)
            nc.sync.dma_start(out=qn_col, in_=qnw)
            kn_col = persist.tile([P, 1], f32r)
            nc.sync.dma_start(out=kn_col, in_=knw)

            # ---- persistent activations ----
            QT = persist.tile([P, NHC, QL], f32r)    # Q'^T  [d, head, q]
            KTt = persist.tile([P, NKVC, KL], f32r)  # K'^T  [d, kvh, k]

            def norm_rope(proj_ps, w_col, cosc, sinc, dst, midp, psums):
                """proj_ps [128,512] PSUM -> dst (f32r SBUF): rmsnorm+rope."""
                ssqp, rotp = psums
                sq = midp.tile([P, 512], f32r, tag="sq")
                nc.scalar.activation(sq, proj_ps, func=AF.Square)
                ssq = ssqp.tile([P, 512], f32, tag="ssq")
                nc.tensor.matmul(ssq, ones128, sq, start=True, stop=True)
                # rstd = 1/sqrt(var+eps); Square/AbsRsqrt/Copy share one
                # table set, so the whole kernel does 2 ACT_TABLE_LOADs.
                rstd = midp.tile([P, 512], f32, tag="rstd")
                nc.scalar.activation(rstd, ssq, func=AF.Abs_reciprocal_sqrt,
                                     scale=1.0 / HD, bias=eps_col)
                qn = midp.tile([P, 512], f32r, tag="qn")
                nc.vector.scalar_tensor_tensor(
                    qn, proj_ps, w_col, rstd,
                    op0=ALU.mult, op1=ALU.mult)
                rot = rotp.tile([P, 512], f32, tag="rot")
                nc.tensor.matmul(rot, rotm, qn, start=True, stop=True)
                t1 = midp.tile([P, 512], f32, tag="sq")
                nc.vector.tensor_mul(t1, qn.bitcast(f32), cosc.bitcast(f32))
                t2 = midp.tile([P, 512], f32, tag="rstd")
                nc.vector.tensor_mul(t2, rot, sinc.bitcast(f32))
                nc.vector.tensor_add(dst, t1, t2)

            # ========= Stage QKV =========
            with tc.tile_pool(name="kv_c", bufs=2) as kvp, \
                 tc.tile_pool(name="kv_cs", bufs=2) as csp2, \
                 tc.tile_pool(name="kv_w", bufs=1) as wp, \
                 tc.tile_pool(name="q_w", bufs=2) as wqp, \
                 tc.tile_pool(name="kv_mid", bufs=2) as midp, \
                 tc.tile_pool(name="kv_vo", bufs=2) as vop, \
                 tc.tile_pool(name="kv_pk", bufs=2, space="PSUM") as pkp, \
                 tc.tile_pool(name="kv_pv", bufs=2, space="PSUM") as pvp, \
                 tc.tile_pool(name="kv_ssq", bufs=2, space="PSUM") as ssqp, \
                 tc.tile_pool(name="kv_rot", bufs=2, space="PSUM") as rotp:
                wk_sb = wp.tile([P, HT, NKVC * HD], f32r)
                nc.sync.dma_start(out=wk_sb, in_=wk)
                wv_sb = wp.tile([P, HT, NKVC * HD], f32r)
                nc.sync.dma_start(out=wv_sb, in_=wv)
                for ch in range(8):
                    sl = slice(ch * 512, (ch + 1) * 512)
                    kvcA = kvp.tile([P, HT // 2, 512], f32r, tag="kvA")
                    nc.sync.dma_start(out=kvcA, in_=kvT[:, 0:HT // 2, sl])
                    kvcB = kvp.tile([P, HT // 2, 512], f32r, tag="kvB")
                    nc.sync.dma_start(out=kvcB, in_=kvT[:, HT // 2:HT, sl])

                    def kvc_at(ht):
                        return (kvcA[:, ht, :] if ht < HT // 2
                                else kvcB[:, ht - HT // 2, :])
                    cosc = csp2.tile([P, 512], f32r, tag="cos")
                    nc.scalar.dma_start(out=cosc, in_=cosT[:, sl])
                    sinc = csp2.tile([P, 512], f32r, tag="sin")
                    nc.scalar.dma_start(out=sinc, in_=sinT[:, sl])
                    # K^T projection + norm + rope (per kv head)
                    for kh in range(NKVC):
                        ps = pkp.tile([P, 512], f32, tag="proj")
                        for ht in range(HT):
                            nc.tensor.matmul(
                                ps, wk_sb[:, ht, kh * HD:(kh + 1) * HD],
                                kvc_at(ht),
                                start=(ht == 0), stop=(ht == HT - 1))
                        norm_rope(ps, kn_col, cosc, sinc,
                                  KTt[:, kh, sl], midp, (ssqp, rotp))
                    # V projection (natural layout), staged to HBM
                    for half in range(2):
                        psv = pvp.tile([P, 2, NKVC * HD], f32, tag="pv")
                        for tt2 in range(2):
                            tt = half * 2 + tt2
                            for ht in range(HT):
                                nc.tensor.matmul(
                                    psv[:, tt2, :],
                                    kvc_at(ht)[:, tt * P:(tt + 1) * P],
                                    wv_sb[:, ht, :],
                                    start=(ht == 0), stop=(ht == HT - 1))
                        v_sb = vop.tile([P, 2, NKVC * HD], f32r, tag="vsb")
                        nc.scalar.activation(v_sb, psv, func=AF.Copy)
                        kt0 = ch * 4 + half * 2
                        nc.scalar.dma_start(out=v_dram[:, kt0:kt0 + 2, :],
                                              in_=v_sb)
                    # Q projection for the noise rows (chunks 6, 7)
                    if ch >= 6:
                        qc = ch - 6
                        for qh in range(NHC):
                            wqt = wqp.tile([P, HT, HD], f32r, tag="wq")
                            nc.scalar.dma_start(out=wqt, in_=wq[:, qh])
                            psq = pkp.tile([P, 512], f32, tag="proj")
                            for ht in range(HT):
                                nc.tensor.matmul(
                                    psq, wqt[:, ht, :], kvc_at(ht),
                                    start=(ht == 0), stop=(ht == HT - 1))
                            norm_rope(psq, qn_col, cosc, sinc,
                                      QT[:, qh, qc * 512:(qc + 1) * 512],
                                      midp, (ssqp, rotp))

            # ================= Stage ATT =================
            with tc.tile_pool(name="ot_persist", bufs=1) as otpp, \
                 tc.tile_pool(name="o_w0", bufs=1) as wopA:
                OT = otpp.tile([P, NHC, QL], f32r)
                w0 = wopA.tile([P, NHC, 1024], f32r)
                nc.sync.dma_start(out=w0, in_=wo[:, 0])
                _stage_att(nc, tc, OT, KTt, QT, v_dram, ones_col)
                _stage_o(nc, tc, OT, wo, out, w0)

    nc.compile()
    return nc


def _stage_att(nc, tc, OT, KTt, QT, v_dram, ones_col):
    with tc.tile_pool(name="at_v", bufs=2) as vp, \
         tc.tile_pool(name="at_et", bufs=3) as etp, \
         tc.tile_pool(name="at_or", bufs=2) as orp, \
         tc.tile_pool(name="at_rec", bufs=2) as recp, \
         tc.tile_pool(name="at_bc", bufs=2) as bcp, \
         tc.tile_pool(name="at_st", bufs=2, space="PSUM") as sTp, \
         tc.tile_pool(name="at_ops", bufs=1, space="PSUM") as oTp, \
         tc.tile_pool(name="at_den", bufs=2, space="PSUM") as denp:
        v_kv = None

        def normalize(lh, oT, dens):
            """Release oT PSUM via ACT copy; fast 1/den (releases den slots
            ~5x sooner than exact reciprocal); gpsimd broadcast; DVE
            multiply. Zero PE involvement, all on idle ATT engines."""
            oraw = orp.tile([P, QL], f32r, tag="oraw", name=f"oraw_{lh}")
            nc.scalar.activation(oraw, oT, func=AF.Copy)
            rec = recp.tile([1, QL], f32, tag="rec", name=f"rec_{lh}")
            nc.vector.reciprocal_approx_fast(out=rec[:, 0:512], in_=dens[0])
            nc.vector.reciprocal_approx_fast(out=rec[:, 512:1024],
                                             in_=dens[1])
            bc = bcp.tile([P, QL], f32, tag="bc", name=f"bc_{lh}")
            nc.gpsimd.partition_broadcast(bc, rec)
            nc.vector.tensor_mul(OT[:, lh, :], oraw, bc.bitcast(f32r))

        def load_v_pieces(kvh):
            """[P, 8, HD] pieces; piece p only depends on kv chunks
            2p..2p+1, so early pieces stream in while later V projections
            (or earlier heads) are still running."""
            tiles = []
            for p in range(4):
                t = vp.tile([P, 8, HD], f32r, tag=f"vk{p}",
                            name=f"vk_{kvh}_{p}")
                nc.sync.dma_start(
                    out=t,
                    in_=v_dram[:, p * 8:(p + 1) * 8,
                               kvh * HD:(kvh + 1) * HD])
                tiles.append(t)
            return tiles

        v_kv = load_v_pieces(0)
        v_next = None
        pend = None  # (eT, dens, oT, kt, v_kv, lh) awaiting den/PV
        for lh in range(NHC):
            kvh = lh // 4
            if lh == 2:
                v_next = load_v_pieces(1)
            if lh == 4:
                v_kv = v_next
            oT = oTp.tile([P, QL], f32, tag="oT")
            dens = [denp.tile([1, 512], f32, tag="den",
                              name=f"den_{lh}_{q}") for q in range(2)]
            for kt in range(KT_N):
                # S^T + exp for this kt (one FD-1024 exp over both q-halves)
                sT = sTp.tile([P, QL], f32, tag="sT")
                for qc in range(2):
                    nc.tensor.matmul(
                        sT[:, qc * 512:(qc + 1) * 512],
                        KTt[:, kvh, kt * P:(kt + 1) * P],
                        QT[:, lh, qc * 512:(qc + 1) * 512],
                        start=True, stop=True)
                eT = etp.tile([P, QL], f32r, tag="eT")
                nc.scalar.activation(eT, sT, func=AF.Exp, scale=SCALE)
                # den/PV for the PREVIOUS kt (software pipeline: PE never
                # waits on the exp it just launched)
                if pend is not None:
                    peT, pdens, poT, pkt, pv, plh = pend
                    pvt = pv[pkt // 8][:, pkt % 8, :]
                    for qc in range(2):
                        nc.tensor.matmul(
                            poT[:, qc * 512:(qc + 1) * 512],
                            pvt,
                            peT[:, qc * 512:(qc + 1) * 512],
                            start=(pkt == 0), stop=(pkt == KT_N - 1))
                    for qc in range(2):
                        nc.tensor.matmul(
                            pdens[qc], ones_col,
                            peT[:, qc * 512:(qc + 1) * 512],
                            start=(pkt == 0), stop=(pkt == KT_N - 1))
                    if pkt == KT_N - 1:
                        normalize(plh, poT, pdens)
                pend = (eT, dens, oT, kt, v_kv, lh)
        # drain the last head
        peT, pdens, poT, pkt, pv, plh = pend
        pvt = pv[pkt // 8][:, pkt % 8, :]
        for qc in range(2):
            nc.tensor.matmul(poT[:, qc * 512:(qc + 1) * 512],
                             pvt,
                             peT[:, qc * 512:(qc + 1) * 512],
                             start=(pkt == 0), stop=(pkt == KT_N - 1))
        for qc in range(2):
            nc.tensor.matmul(pdens[qc], ones_col,
                             peT[:, qc * 512:(qc + 1) * 512],
                             start=(pkt == 0), stop=(pkt == KT_N - 1))
        normalize(plh, poT, pdens)


def _stage_o(nc, tc, OT, wo, out, w0):
    # Two column passes: pass 0 uses pre-loaded w0 (cols 0:1024); pass 1's
    # w1 load overlaps pass 0's matmuls.
    with tc.tile_pool(name="o_w1", bufs=1) as wopB, \
         tc.tile_pool(name="o_out", bufs=3) as outp, \
         tc.tile_pool(name="o_ps", bufs=4, space="PSUM") as opsp:
        w1 = wopB.tile([P, NHC, 1024], f32r)
        nc.sync.dma_start(out=w1, in_=wo[:, 1])
        for half, wsb in ((0, w0), (1, w1)):
            for qt in range(8):
                ob = outp.tile([P, 1024], f32, tag="ob")
                pss = [opsp.tile([P, 512], f32, tag="ops",
                                 name=f"ops_{half}_{qt}_{i}") for i in range(2)]
                for ci in range(NHC):
                    for nch in range(2):
                        nc.tensor.matmul(
                            pss[nch], OT[:, ci, qt * P:(qt + 1) * P],
                            wsb[:, ci, nch * 512:(nch + 1) * 512],
                            start=(ci == 0), stop=(ci == NHC - 1))
                for nch in range(2):
                    nc.any.tensor_copy(ob[:, nch * 512:(nch + 1) * 512],
                                       pss[nch])
                nc.scalar.dma_start(
                    out=out[qt * P:(qt + 1) * P,
                            half * 1024:(half + 1) * 1024],
                    in_=ob)


def _get_nc():
    global _NC
    if _NC is None:
        _NC = build_nc()
    return _NC


def _make_in_maps(noise, ctx, cos, sin, Wq, Wk, Wv, Wo, qn_w, kn_w):
    noise = np.asarray(noise, np.float32)
    ctx = np.asarray(ctx, np.float32)
    cos = np.asarray(cos, np.float32)
    sin = np.asarray(sin, np.float32)
    Wq = np.asarray(Wq, np.float32)
    Wk = np.asarray(Wk, np.float32)
    Wv = np.asarray(Wv, np.float32)
    Wo = np.asarray(Wo, np.float32)
    qn_w = np.asarray(qn_w, np.float32).reshape(P, 1)
    kn_w = np.asarray(kn_w, np.float32).reshape(P, 1)
    B = noise.shape[0]
    in_maps = []
    for b in range(B):
        kv_b = np.concatenate([ctx[b], noise[b]], axis=0)      # [KL, H]
        # [p, ht, t]
        kvT_b = np.ascontiguousarray(
            kv_b.T.reshape(HT, P, KL).transpose(1, 0, 2))
        cosT_b = np.ascontiguousarray(cos[b].T)                 # [P, KL]
        sinT_b = np.ascontiguousarray(sin[b].T)
        for g in range(2):
            Wq_g = Wq[:, g * 1024:(g + 1) * 1024]
            Wk_g = Wk[:, g * 256:(g + 1) * 256]
            Wv_g = Wv[:, g * 256:(g + 1) * 256]
            Wo_g = Wo[g * 1024:(g + 1) * 1024, :]
            in_maps.append({
                "kvT": kvT_b,
                "cosT": cosT_b,
                "sinT": sinT_b,
                "wq": np.ascontiguousarray(
                    Wq_g.reshape(HT, P, NHC, HD).transpose(1, 2, 0, 3)),
                "wk": np.ascontiguousarray(
                    Wk_g.reshape(HT, P, NKVC * HD).transpose(1, 0, 2)),
                "wv": np.ascontiguousarray(
                    Wv_g.reshape(HT, P, NKVC * HD).transpose(1, 0, 2)),
                "wo": np.ascontiguousarray(
                    Wo_g.reshape(NHC, P, 2, 1024).transpose(1, 2, 0, 3)),
                "qnw": qn_w,
                "knw": kn_w,
            })
    return in_maps


def _install_profile_hook():
    """Provide antenv.axon_hooks (absent in this container) so
    run_bass_kernel_spmd(trace=True) can NTFF-profile via libaxon_pjrt."""
    import types
    if "antenv.axon_hooks" not in sys.modules:
        import antenv
        mod = types.ModuleType("antenv.axon_hooks")
        _state = {}
        mod.set_axon_ntff_profile_hook = lambda h: _state.__setitem__("h", h)
        mod.get_axon_ntff_profile_hook = lambda: _state.get("h")
        sys.modules["antenv.axon_hooks"] = mod
        antenv.axon_hooks = mod
        from trn_agent_boot.trn_boot import _ntff_profile_via_ctypes
        mod.set_axon_ntff_profile_hook(
            _ntff_profile_via_ctypes("/opt/axon/libaxon_pjrt.so"))
    import concourse.bass_utils as bu
    bu.upload_artifacts = lambda tmpdir: tmpdir


def run(inputs, trace=False, tmpdir=None):
    """Run on 8 cores; returns (output [4,1024,2048], results)."""
    nc = _get_nc()
    in_maps = _make_in_maps(**inputs)
    if trace:
        _install_profile_hook()
    res = run_bass_kernel_spmd(nc, in_maps, core_ids=list(range(8)),
                               trace=trace, tmpdir=tmpdir,
                               trace_cores=[0] if trace else None)
    outs = [res.results[i]["out"] for i in range(8)]
    full = np.stack([outs[2 * b] + outs[2 * b + 1] for b in range(4)], axis=0)
    return full.astype(np.float32), res


def kernel(**inputs):
    out, _ = run(inputs, trace=False)
    return out


def _attr(i, name):
    v = getattr(i, name, None)
    try:
        if callable(v):
            v = v()
    except Exception:
        v = "?"
    return v


def summarize_trace(res, top=30):
    """Per-engine busy time + per-op totals."""
    if not res.instructions_and_trace:
        print("no trace")
        return
    insts, trace_path = res.instructions_and_trace
    from collections import defaultdict
    eng_busy = defaultdict(int)
    eng_n = defaultdict(int)
    op_cost = defaultdict(int)
    op_n = defaultdict(int)
    t0 = min(i.timestamp for i in insts)
    t1 = max(i.end_timestamp for i in insts)
    for i in insts:
        eng = str(_attr(i, "engine"))
        op = str(_attr(i, "op_name"))
        eng_busy[eng] += i.duration
        eng_n[eng] += 1
        op_cost[(eng, op)] += i.duration
        op_n[(eng, op)] += 1
    span = t1 - t0
    print(f"trace: {trace_path}")
    print(f"span: {span} ns")
    for e in sorted(eng_busy, key=lambda e: -eng_busy[e]):
        print(f"  {e:14s} busy {eng_busy[e]:>10} ns "
              f"({100.0*eng_busy[e]/span:5.1f}%)  n={eng_n[e]}")
    print("per (engine, op): total_ns, n, avg_ns")
    for (e, op), c in sorted(op_cost.items(), key=lambda kv: -kv[1])[:top]:
        n = op_n[(e, op)]
        print(f"  {c:>10} ns  n={n:<5d} avg={c//max(n,1):>7} ns  {e:12s} {op}")
